# revision 16
# baseline (speedup 1.0000x reference)
"""DBRX MoE experts kernel for Trainium2 (8 NeuronCores).

Strategy (expert-parallel, all-bf16, zero collectives):
  - Router (logits -> softmax -> top-2 -> renormalize) computed on host in
    numpy (0.01% of FLOPs); it determines the token->expert dispatch.
  - Core c owns expert c entirely: full gate/up [2I, D] and down [D, I]
    weights, processing only that expert's tokens (padded to the max expert
    count NPT so all 8 cores run the identical SPMD program).
  - Everything is bf16 (weights, tokens, h, output); PSUM accumulates fp32.
    bf16 keeps the tensor engine at 1 cycle/row for any moving-dim size
    (no fp32r >=256 free-dim constraint) and halves HBM traffic.
  - Phase A (gate/up): for each 128-row I-chunk, stream that weight chunk
    once; tokens are the moving dim in 512-wide blocks (PSUM bank = 512
    fp32). h = silu(gate) * up is written bf16 and stays resident in SBUF.
  - Phase B (down): for each 128-row D-chunk, stream the w2 chunk once;
    accumulate over all 32 I-chunks into one PSUM bank; evacuate bf16 and
    DMA out. Output rows are the *unweighted* FFN outputs.
  - No collective: each token's FFN output is computed entirely by one core.
    The host scales the two expert contributions by the renormalized top-2
    weights and sums them (fp32).
"""

import numpy as np

T = 4096
D = 2048
E = 8
I = 4096
NCORES = 8
P = 128
DCH = D // P  # 16 d-chunks
ICH = I // P  # 32 i-chunks
BLK = 512  # token block (PSUM bank = 512 fp32)


def _host_router(x, router_w):
    """Replicate reference routing in numpy (fp32)."""
    logits = (x.astype(np.float64) @ router_w.astype(np.float64).T).astype(np.float32)
    m = logits.max(axis=-1, keepdims=True)
    ex = np.exp((logits - m).astype(np.float32))
    probs = ex / ex.sum(axis=-1, keepdims=True)
    # top-2, ties to lower index (matches jax.lax.top_k)
    top1 = probs.argmax(axis=-1)
    p = probs.copy()
    p[np.arange(T), top1] = -1.0
    top2 = p.argmax(axis=-1)
    w1 = probs[np.arange(T), top1]
    w2 = probs[np.arange(T), top2]
    s = w1 + w2
    return top1.astype(np.int64), top2.astype(np.int64), (w1 / s).astype(np.float32), (w2 / s).astype(np.float32)


_CACHE: dict = {}


def _blocks(npt):
    """Standard block list, tail block FIRST (the tiny tail between two big
    blocks would otherwise make PSUM-bank reuse waits land on the critical
    path at every outer-loop boundary)."""
    out = []
    t0 = 0
    while t0 < npt:
        n = min(BLK, npt - t0)
        out.append((t0, n))
        t0 += n
    if len(out) > 1 and out[-1][1] < BLK:
        out = [out[-1]] + out[:-1]
    return out


def _startup_blocks(npt):
    """Finer-grained block list for the first I-chunk so the first matmuls
    only wait on small token DMAs. Covers the same [0, npt) as _blocks, in
    ascending order with the tail LAST (chunk arrival order)."""
    out = []
    for i, (t0, n) in enumerate(sorted(_blocks(npt))):
        if n == BLK and i == 0:
            out.extend((t0 + off, 128) for off in range(0, BLK, 128))
        elif n == BLK:
            out.extend((t0 + off, 256) for off in range(0, BLK, 256))
        else:
            out.append((t0, n))
    return out


def _build_bass(npt: int):
    """Build the single SPMD Bass program: one expert's FFN over npt packed
    token slots."""
    import concourse.bacc as bacc
    import concourse.mybir as mybir
    import concourse.tile as tile

    f32 = mybir.dt.float32
    bf16 = mybir.dt.bfloat16
    blocks = _blocks(npt)
    blocks0 = _startup_blocks(npt)

    nc = bacc.Bacc("TRN2", target_bir_lowering=False)

    xt_d = nc.dram_tensor("xt", [P, DCH, npt], bf16, kind="ExternalInput")
    wst_d = nc.dram_tensor("wst", [ICH, P, 2 * DCH * P], bf16, kind="ExternalInput")
    w2t_d = nc.dram_tensor("w2t", [DCH, P, ICH * P], bf16, kind="ExternalInput")
    out_d = nc.dram_tensor("out", [DCH, P, npt], bf16, kind="ExternalOutput")

    with tile.TileContext(nc) as tc:
        with (
            tc.tile_pool(name="xpool", bufs=1) as xpool,
            tc.tile_pool(name="hpool", bufs=1) as hpool,
            tc.tile_pool(name="wpool", bufs=4) as wpool,
            tc.tile_pool(name="w2pool", bufs=3) as w2pool,
            tc.tile_pool(name="spool", bufs=6) as spool,
            tc.tile_pool(name="opool", bufs=3) as opool,
            tc.tile_pool(name="ps", bufs=8, space="PSUM") as ps_pool,
        ):
            xt_sb = xpool.tile([P, DCH, npt], bf16)
            hT = hpool.tile([P, ICH, npt], bf16)

            # Startup: gate half, up half, then token chunks smallest-first in
            # ascending order. ic=0 walks blocks0 in the same order, so every
            # dependency lands just before PE reaches it.
            wg0 = wpool.tile([P, DCH * P], bf16, tag="wg")
            nc.sync.dma_start(wg0[:], wst_d[0][:, : DCH * P])
            wu0 = wpool.tile([P, DCH * P], bf16, tag="wu")
            nc.sync.dma_start(wu0[:], wst_d[0][:, DCH * P :])
            for t0, n in blocks0:
                nc.sync.dma_start(xt_sb[:, :, t0 : t0 + n], xt_d[:, :, t0 : t0 + n])

            # ---- phase A: gate/up + SwiGLU, h resident ----
            for ic in range(ICH):
                if ic == 0:
                    wg, wu = wg0, wu0
                else:
                    wg = wpool.tile([P, DCH * P], bf16, tag="wg")
                    nc.sync.dma_start(wg[:], wst_d[ic][:, : DCH * P])
                    wu = wpool.tile([P, DCH * P], bf16, tag="wu")
                    nc.sync.dma_start(wu[:], wst_d[ic][:, DCH * P :])
                for t0, n in blocks0 if ic == 0 else blocks:
                    pg = ps_pool.tile([P, BLK], f32, tag="ps", name=f"pg_{ic}_{t0}")
                    pu = ps_pool.tile([P, BLK], f32, tag="ps", name=f"pu_{ic}_{t0}")
                    for dc in range(DCH):
                        nc.tensor.matmul(
                            pg[:, :n],
                            wg[:, dc * P : (dc + 1) * P],
                            xt_sb[:, dc, t0 : t0 + n],
                            start=(dc == 0),
                            stop=(dc == DCH - 1),
                        )
                        nc.tensor.matmul(
                            pu[:, :n],
                            wu[:, dc * P : (dc + 1) * P],
                            xt_sb[:, dc, t0 : t0 + n],
                            start=(dc == 0),
                            stop=(dc == DCH - 1),
                        )
                    sg = spool.tile([P, BLK], bf16, tag="sg")
                    nc.scalar.activation(
                        sg[:, :n], pg[:, :n], mybir.ActivationFunctionType.Silu
                    )
                    nc.vector.tensor_mul(hT[:, ic, t0 : t0 + n], sg[:, :n], pu[:, :n])

            # ---- phase B: down proj ----
            for dc in range(DCH):
                w2 = w2pool.tile([P, ICH * P], bf16, tag="w2t")
                nc.sync.dma_start(w2[:], w2t_d[dc])
                # last d-chunk: tiny tail block last, so the final output DMA
                # (on the critical path) is the smallest one
                blk_b = blocks if dc < DCH - 1 else (blocks[1:] + blocks[:1])
                for t0, n in blk_b:
                    po = ps_pool.tile([P, BLK], f32, tag="ps", name=f"po_{dc}_{t0}")
                    for ic in range(ICH):
                        nc.tensor.matmul(
                            po[:, :n],
                            w2[:, ic * P : (ic + 1) * P],
                            hT[:, ic, t0 : t0 + n],
                            start=(ic == 0),
                            stop=(ic == ICH - 1),
                        )
                    ob = opool.tile([P, BLK], bf16, tag="ob")
                    nc.scalar.activation(
                        ob[:, :n], po[:, :n], mybir.ActivationFunctionType.Copy
                    )
                    nc.sync.dma_start(out_d[dc, :, t0 : t0 + n], ob[:, :n])

    nc.compile()
    return nc


def _prepare(hidden_states, router_w, ws, w2s):
    """Host-side routing, per-expert packing, weight transposes, bf16 casts."""
    import ml_dtypes

    bf16 = ml_dtypes.bfloat16

    x = np.asarray(hidden_states, dtype=np.float32).reshape(T, D)
    router_w = np.asarray(router_w, dtype=np.float32)
    ws = np.asarray(ws, dtype=np.float32)
    w2s = np.asarray(w2s, dtype=np.float32)

    top1, top2, w1, w2 = _host_router(x, router_w)

    # per-expert token lists; pos[k, t] = global packed position of token t's
    # top-k contribution (k=0 -> top1 expert, k=1 -> top2 expert)
    toks: list[list[int]] = [[] for _ in range(E)]
    loc = np.zeros((2, T), dtype=np.int64)
    exp = np.zeros((2, T), dtype=np.int64)
    for k, ti in enumerate((top1, top2)):
        for t in range(T):
            e = int(ti[t])
            loc[k, t] = len(toks[e])
            exp[k, t] = e
            toks[e].append(t)
    cnts = [len(tk) for tk in toks]
    npt = max(cnts)
    base = np.zeros(E + 1, dtype=np.int64)
    base[1:] = np.cumsum(cnts)
    pos = base[exp] + loc  # [2, T]

    x_bf = x.astype(bf16)
    xt_all = []
    for e in range(E):
        cnt = cnts[e]
        xe = np.zeros((npt, DCH, P), dtype=bf16)
        xe[:cnt] = x_bf[toks[e]].reshape(cnt, DCH, P)
        xt_all.append(np.ascontiguousarray(xe.transpose(2, 1, 0)))  # [P, DCH, npt]

    wst_all = []
    w2t_all = []
    for e in range(E):
        gate = ws[e, :I, :]  # [I, D]
        up = ws[e, I:, :]
        # [ICH, P(d), DCH, P(m=i)]
        gt = gate.reshape(ICH, P, DCH, P).transpose(0, 3, 2, 1)
        ut = up.reshape(ICH, P, DCH, P).transpose(0, 3, 2, 1)
        wst = np.stack([gt, ut], axis=2)  # [ICH, P, 2, DCH, P]
        wst_all.append(
            np.ascontiguousarray(wst.reshape(ICH, P, 2 * DCH * P).astype(bf16))
        )
        # w2s[e]: [D, I] -> [DCH, P(i), ICH, P(m=d)]
        w2t = w2s[e].reshape(DCH, P, ICH, P).transpose(0, 3, 2, 1)
        w2t_all.append(
            np.ascontiguousarray(w2t.reshape(DCH, P, ICH * P).astype(bf16))
        )

    return cnts, npt, pos, (w1, w2), xt_all, wst_all, w2t_all


def kernel(hidden_states, router_w, ws, w2s):
    from concourse import bass_utils

    hs = np.asarray(hidden_states)
    B, S, _ = hs.shape
    cnts, npt, pos, (w1, w2), xt_all, wst_all, w2t_all = _prepare(
        hidden_states, router_w, ws, w2s
    )

    if npt not in _CACHE:
        _CACHE[npt] = _build_bass(npt)
    nc = _CACHE[npt]

    in_maps = [
        {"xt": xt_all[c], "wst": wst_all[c], "w2t": w2t_all[c]} for c in range(NCORES)
    ]
    res = bass_utils.run_bass_kernel_spmd(nc, in_maps, core_ids=list(range(NCORES)))

    # assemble: per-core out [DCH, P, npt] -> packed [sum(cnts), D]
    packed = np.empty((sum(cnts), D), dtype=np.float32)
    base = 0
    for e in range(E):
        oc = np.asarray(res.results[e]["out"]).astype(np.float32)  # [DCH, P, npt]
        packed[base : base + cnts[e]] = (
            oc[:, :, : cnts[e]].reshape(D, cnts[e]).T
        )
        base += cnts[e]

    out = w1[:, None] * packed[pos[0]] + w2[:, None] * packed[pos[1]]
    return out.reshape(B, S, D).astype(np.float32)


# revision 21
# speedup vs baseline: 1.0043x; 1.0043x over previous
"""DBRX MoE experts kernel for Trainium2 (8 NeuronCores).

Strategy (expert-parallel, all-bf16, zero collectives):
  - Router (logits -> softmax -> top-2 -> renormalize) computed on host in
    numpy (0.01% of FLOPs); it determines the token->expert dispatch.
  - Core c owns expert c entirely: full gate/up [2I, D] and down [D, I]
    weights, processing only that expert's tokens (padded to the max expert
    count NPT so all 8 cores run the identical SPMD program).
  - Everything is bf16 (weights, tokens, h, output); PSUM accumulates fp32.
    bf16 keeps the tensor engine at 1 cycle/row for any moving-dim size
    (no fp32r >=256 free-dim constraint) and halves HBM traffic.
  - Phase A (gate/up): for each 128-row I-chunk, stream that weight chunk
    once; tokens are the moving dim in 512-wide blocks (PSUM bank = 512
    fp32). h = silu(gate) * up is written bf16 and stays resident in SBUF.
  - Phase B (down): for each 128-row D-chunk, stream the w2 chunk once;
    accumulate over all 32 I-chunks into one PSUM bank; evacuate bf16 and
    DMA out. Output rows are the *unweighted* FFN outputs.
  - No collective: each token's FFN output is computed entirely by one core.
    The host scales the two expert contributions by the renormalized top-2
    weights and sums them (fp32).
"""

import numpy as np

T = 4096
D = 2048
E = 8
I = 4096
NCORES = 8
P = 128
DCH = D // P  # 16 d-chunks
ICH = I // P  # 32 i-chunks
BLK = 512  # token block (PSUM bank = 512 fp32)


def _host_router(x, router_w):
    """Replicate reference routing in numpy (fp32)."""
    logits = (x.astype(np.float64) @ router_w.astype(np.float64).T).astype(np.float32)
    m = logits.max(axis=-1, keepdims=True)
    ex = np.exp((logits - m).astype(np.float32))
    probs = ex / ex.sum(axis=-1, keepdims=True)
    # top-2, ties to lower index (matches jax.lax.top_k)
    top1 = probs.argmax(axis=-1)
    p = probs.copy()
    p[np.arange(T), top1] = -1.0
    top2 = p.argmax(axis=-1)
    w1 = probs[np.arange(T), top1]
    w2 = probs[np.arange(T), top2]
    s = w1 + w2
    return top1.astype(np.int64), top2.astype(np.int64), (w1 / s).astype(np.float32), (w2 / s).astype(np.float32)


_CACHE: dict = {}


def _blocks(npt):
    """Standard block list, tail block FIRST (the tiny tail between two big
    blocks would otherwise make PSUM-bank reuse waits land on the critical
    path at every outer-loop boundary)."""
    out = []
    t0 = 0
    while t0 < npt:
        n = min(BLK, npt - t0)
        out.append((t0, n))
        t0 += n
    if len(out) > 1 and out[-1][1] < BLK:
        out = [out[-1]] + out[:-1]
    return out


def _startup_blocks(npt):
    """Finer-grained block list for the first I-chunk so the first matmuls
    only wait on small token DMAs. Covers the same [0, npt) as _blocks, in
    ascending order with the tail LAST (chunk arrival order)."""
    out = []
    first = True
    for t0, n in sorted(_blocks(npt)):
        if n == BLK and first:
            out.extend(
                (t0 + off, m)
                for off, m in ((0, 64), (64, 64), (128, 128), (256, 256))
            )
            first = False
        elif n == BLK:
            out.extend((t0 + off, 256) for off in range(0, BLK, 256))
        else:
            out.append((t0, n))
    return out


def _build_bass(npt: int):
    """Build the single SPMD Bass program: one expert's FFN over npt packed
    token slots."""
    import concourse.bacc as bacc
    import concourse.mybir as mybir
    import concourse.tile as tile

    f32 = mybir.dt.float32
    bf16 = mybir.dt.bfloat16
    blocks = _blocks(npt)
    blocks0 = _startup_blocks(npt)

    nc = bacc.Bacc("TRN2", target_bir_lowering=False)

    # chunk-major token layout: block (t0, n) occupies cols [DCH*t0, DCH*(t0+n))
    # as [dc, tok] so every chunk DMA is fully contiguous (full DMA bandwidth
    # at any chunk size)
    xt_d = nc.dram_tensor("xt", [P, DCH * npt], bf16, kind="ExternalInput")
    wst_d = nc.dram_tensor("wst", [ICH, P, 2 * DCH * P], bf16, kind="ExternalInput")
    w2t_d = nc.dram_tensor("w2t", [DCH, P, ICH * P], bf16, kind="ExternalInput")
    out_d = nc.dram_tensor("out", [DCH, P, npt], bf16, kind="ExternalOutput")

    with tile.TileContext(nc) as tc:
        with (
            tc.tile_pool(name="xpool", bufs=1) as xpool,
            tc.tile_pool(name="hpool", bufs=1) as hpool,
            tc.tile_pool(name="wpool", bufs=4) as wpool,
            tc.tile_pool(name="w2pool", bufs=3) as w2pool,
            tc.tile_pool(name="spool", bufs=6) as spool,
            tc.tile_pool(name="opool", bufs=3) as opool,
            tc.tile_pool(name="ps", bufs=8, space="PSUM") as ps_pool,
        ):
            hT = hpool.tile([P, ICH, npt], bf16)

            # Startup: gate half, up half, then token chunks smallest-first in
            # ascending order. Phase A walks blocks0 in the same order, so
            # every dependency lands just before PE reaches it and the PE
            # never stalls (an idle PE resets the clock ramp to half speed).
            wg0 = wpool.tile([P, DCH * P], bf16, tag="wg")
            nc.sync.dma_start(wg0[:], wst_d[0][:, : DCH * P])
            wu0 = wpool.tile([P, DCH * P], bf16, tag="wu")
            nc.sync.dma_start(wu0[:], wst_d[0][:, DCH * P :])
            xb = {}
            for t0, n in blocks0:
                xt = xpool.tile([P, DCH, n], bf16, tag=f"xb{t0}")
                xb[t0] = xt
                nc.sync.dma_start(xt[:], xt_d[:, DCH * t0 : DCH * (t0 + n)])

            # ---- phase A: gate/up + SwiGLU, h resident ----
            for ic in range(ICH):
                if ic == 0:
                    wg, wu = wg0, wu0
                else:
                    wg = wpool.tile([P, DCH * P], bf16, tag="wg")
                    nc.sync.dma_start(wg[:], wst_d[ic][:, : DCH * P])
                    wu = wpool.tile([P, DCH * P], bf16, tag="wu")
                    nc.sync.dma_start(wu[:], wst_d[ic][:, DCH * P :])
                for t0, n in blocks0:
                    xt = xb[t0]
                    pg = ps_pool.tile([P, BLK], f32, tag="ps", name=f"pg_{ic}_{t0}")
                    pu = ps_pool.tile([P, BLK], f32, tag="ps", name=f"pu_{ic}_{t0}")
                    for dc in range(DCH):
                        nc.tensor.matmul(
                            pg[:, :n],
                            wg[:, dc * P : (dc + 1) * P],
                            xt[:, dc, :],
                            start=(dc == 0),
                            stop=(dc == DCH - 1),
                        )
                        nc.tensor.matmul(
                            pu[:, :n],
                            wu[:, dc * P : (dc + 1) * P],
                            xt[:, dc, :],
                            start=(dc == 0),
                            stop=(dc == DCH - 1),
                        )
                    sg = spool.tile([P, BLK], bf16, tag="sg")
                    nc.scalar.activation(
                        sg[:, :n], pg[:, :n], mybir.ActivationFunctionType.Silu
                    )
                    nc.vector.tensor_mul(hT[:, ic, t0 : t0 + n], sg[:, :n], pu[:, :n])

            # ---- phase B: down proj ----
            for dc in range(DCH):
                w2 = w2pool.tile([P, ICH * P], bf16, tag="w2t")
                nc.sync.dma_start(w2[:], w2t_d[dc])
                # last d-chunk: tiny tail block last, so the final output DMA
                # (on the critical path) is the smallest one
                blk_b = blocks if dc < DCH - 1 else (blocks[1:] + blocks[:1])
                for t0, n in blk_b:
                    po = ps_pool.tile([P, BLK], f32, tag="ps", name=f"po_{dc}_{t0}")
                    for ic in range(ICH):
                        nc.tensor.matmul(
                            po[:, :n],
                            w2[:, ic * P : (ic + 1) * P],
                            hT[:, ic, t0 : t0 + n],
                            start=(ic == 0),
                            stop=(ic == ICH - 1),
                        )
                    ob = opool.tile([P, BLK], bf16, tag="ob")
                    nc.scalar.activation(
                        ob[:, :n], po[:, :n], mybir.ActivationFunctionType.Copy
                    )
                    nc.sync.dma_start(out_d[dc, :, t0 : t0 + n], ob[:, :n])

    nc.compile()
    return nc


def _prepare(hidden_states, router_w, ws, w2s):
    """Host-side routing, per-expert packing, weight transposes, bf16 casts."""
    import ml_dtypes

    bf16 = ml_dtypes.bfloat16

    x = np.asarray(hidden_states, dtype=np.float32).reshape(T, D)
    router_w = np.asarray(router_w, dtype=np.float32)
    ws = np.asarray(ws, dtype=np.float32)
    w2s = np.asarray(w2s, dtype=np.float32)

    top1, top2, w1, w2 = _host_router(x, router_w)

    # per-expert token lists; pos[k, t] = global packed position of token t's
    # top-k contribution (k=0 -> top1 expert, k=1 -> top2 expert)
    toks: list[list[int]] = [[] for _ in range(E)]
    loc = np.zeros((2, T), dtype=np.int64)
    exp = np.zeros((2, T), dtype=np.int64)
    for k, ti in enumerate((top1, top2)):
        for t in range(T):
            e = int(ti[t])
            loc[k, t] = len(toks[e])
            exp[k, t] = e
            toks[e].append(t)
    cnts = [len(tk) for tk in toks]
    npt = max(cnts)
    base = np.zeros(E + 1, dtype=np.int64)
    base[1:] = np.cumsum(cnts)
    pos = base[exp] + loc  # [2, T]

    x_bf = x.astype(bf16)
    sblocks = _startup_blocks(npt)
    xt_all = []
    for e in range(E):
        cnt = cnts[e]
        xe = np.zeros((npt, DCH, P), dtype=bf16)
        xe[:cnt] = x_bf[toks[e]].reshape(cnt, DCH, P)
        # chunk-major: block (t0, n) -> cols [DCH*t0, DCH*(t0+n)) as [dc, tok]
        xtc = np.empty((P, DCH * npt), dtype=bf16)
        for t0, n in sblocks:
            xtc[:, DCH * t0 : DCH * (t0 + n)] = (
                xe[t0 : t0 + n].transpose(2, 1, 0).reshape(P, DCH * n)
            )
        xt_all.append(xtc)

    wst_all = []
    w2t_all = []
    for e in range(E):
        gate = ws[e, :I, :]  # [I, D]
        up = ws[e, I:, :]
        # [ICH, P(d), DCH, P(m=i)]
        gt = gate.reshape(ICH, P, DCH, P).transpose(0, 3, 2, 1)
        ut = up.reshape(ICH, P, DCH, P).transpose(0, 3, 2, 1)
        wst = np.stack([gt, ut], axis=2)  # [ICH, P, 2, DCH, P]
        wst_all.append(
            np.ascontiguousarray(wst.reshape(ICH, P, 2 * DCH * P).astype(bf16))
        )
        # w2s[e]: [D, I] -> [DCH, P(i), ICH, P(m=d)]
        w2t = w2s[e].reshape(DCH, P, ICH, P).transpose(0, 3, 2, 1)
        w2t_all.append(
            np.ascontiguousarray(w2t.reshape(DCH, P, ICH * P).astype(bf16))
        )

    return cnts, npt, pos, (w1, w2), xt_all, wst_all, w2t_all


def kernel(hidden_states, router_w, ws, w2s):
    from concourse import bass_utils

    hs = np.asarray(hidden_states)
    B, S, _ = hs.shape
    cnts, npt, pos, (w1, w2), xt_all, wst_all, w2t_all = _prepare(
        hidden_states, router_w, ws, w2s
    )

    if npt not in _CACHE:
        _CACHE[npt] = _build_bass(npt)
    nc = _CACHE[npt]

    in_maps = [
        {"xt": xt_all[c], "wst": wst_all[c], "w2t": w2t_all[c]} for c in range(NCORES)
    ]
    res = bass_utils.run_bass_kernel_spmd(nc, in_maps, core_ids=list(range(NCORES)))

    # assemble: per-core out [DCH, P, npt] -> packed [sum(cnts), D]
    packed = np.empty((sum(cnts), D), dtype=np.float32)
    base = 0
    for e in range(E):
        oc = np.asarray(res.results[e]["out"]).astype(np.float32)  # [DCH, P, npt]
        packed[base : base + cnts[e]] = (
            oc[:, :, : cnts[e]].reshape(D, cnts[e]).T
        )
        base += cnts[e]

    out = w1[:, None] * packed[pos[0]] + w2[:, None] * packed[pos[1]]
    return out.reshape(B, S, D).astype(np.float32)


# revision 23
# speedup vs baseline: 1.0071x; 1.0027x over previous
"""DBRX MoE experts kernel for Trainium2 (8 NeuronCores).

Strategy (expert-parallel, all-bf16, zero collectives):
  - Router (logits -> softmax -> top-2 -> renormalize) computed on host in
    numpy (0.01% of FLOPs); it determines the token->expert dispatch.
  - Core c owns expert c entirely: full gate/up [2I, D] and down [D, I]
    weights, processing only that expert's tokens (padded to the max expert
    count NPT so all 8 cores run the identical SPMD program).
  - Everything is bf16 (weights, tokens, h, output); PSUM accumulates fp32.
    bf16 keeps the tensor engine at 1 cycle/row for any moving-dim size
    (no fp32r >=256 free-dim constraint) and halves HBM traffic.
  - Phase A (gate/up): for each 128-row I-chunk, stream that weight chunk
    once; tokens are the moving dim in 512-wide blocks (PSUM bank = 512
    fp32). h = silu(gate) * up is written bf16 and stays resident in SBUF.
  - Phase B (down): for each 128-row D-chunk, stream the w2 chunk once;
    accumulate over all 32 I-chunks into one PSUM bank; evacuate bf16 and
    DMA out. Output rows are the *unweighted* FFN outputs.
  - No collective: each token's FFN output is computed entirely by one core.
    The host scales the two expert contributions by the renormalized top-2
    weights and sums them (fp32).
"""

import numpy as np

T = 4096
D = 2048
E = 8
I = 4096
NCORES = 8
P = 128
DCH = D // P  # 16 d-chunks
ICH = I // P  # 32 i-chunks
BLK = 512  # token block (PSUM bank = 512 fp32)


def _host_router(x, router_w):
    """Replicate reference routing in numpy (fp32)."""
    logits = (x.astype(np.float64) @ router_w.astype(np.float64).T).astype(np.float32)
    m = logits.max(axis=-1, keepdims=True)
    ex = np.exp((logits - m).astype(np.float32))
    probs = ex / ex.sum(axis=-1, keepdims=True)
    # top-2, ties to lower index (matches jax.lax.top_k)
    top1 = probs.argmax(axis=-1)
    p = probs.copy()
    p[np.arange(T), top1] = -1.0
    top2 = p.argmax(axis=-1)
    w1 = probs[np.arange(T), top1]
    w2 = probs[np.arange(T), top2]
    s = w1 + w2
    return top1.astype(np.int64), top2.astype(np.int64), (w1 / s).astype(np.float32), (w2 / s).astype(np.float32)


_CACHE: dict = {}


def _blocks(npt):
    """Standard block list, tail block FIRST (the tiny tail between two big
    blocks would otherwise make PSUM-bank reuse waits land on the critical
    path at every outer-loop boundary)."""
    out = []
    t0 = 0
    while t0 < npt:
        n = min(BLK, npt - t0)
        out.append((t0, n))
        t0 += n
    if len(out) > 1 and out[-1][1] < BLK:
        out = [out[-1]] + out[:-1]
    return out


def _startup_blocks(npt):
    """Finer-grained block list for the first I-chunk so the first matmuls
    only wait on small token DMAs. Covers the same [0, npt) as _blocks, in
    ascending order with the tail LAST (chunk arrival order)."""
    out = []
    first = True
    for t0, n in sorted(_blocks(npt)):
        if n == BLK and first:
            out.extend(
                (t0 + off, m)
                for off, m in ((0, 64), (64, 64), (128, 128), (256, 128), (384, 128))
            )
            first = False
        elif n == BLK:
            out.extend((t0 + off, 256) for off in range(0, BLK, 256))
        else:
            out.append((t0, n))
    return out


def _build_bass(npt: int):
    """Build the single SPMD Bass program: one expert's FFN over npt packed
    token slots."""
    import concourse.bacc as bacc
    import concourse.mybir as mybir
    import concourse.tile as tile

    f32 = mybir.dt.float32
    bf16 = mybir.dt.bfloat16
    blocks = _blocks(npt)
    blocks0 = _startup_blocks(npt)

    nc = bacc.Bacc("TRN2", target_bir_lowering=False)

    # chunk-major token layout: block (t0, n) occupies cols [DCH*t0, DCH*(t0+n))
    # as [dc, tok] so every chunk DMA is fully contiguous (full DMA bandwidth
    # at any chunk size)
    xt_d = nc.dram_tensor("xt", [P, DCH * npt], bf16, kind="ExternalInput")
    wst_d = nc.dram_tensor("wst", [ICH, P, 2 * DCH * P], bf16, kind="ExternalInput")
    w2t_d = nc.dram_tensor("w2t", [DCH, P, ICH * P], bf16, kind="ExternalInput")
    out_d = nc.dram_tensor("out", [DCH, P, npt], bf16, kind="ExternalOutput")

    with tile.TileContext(nc) as tc:
        with (
            tc.tile_pool(name="xpool", bufs=1) as xpool,
            tc.tile_pool(name="hpool", bufs=1) as hpool,
            tc.tile_pool(name="wpool", bufs=4) as wpool,
            tc.tile_pool(name="w2pool", bufs=3) as w2pool,
            tc.tile_pool(name="spool", bufs=6) as spool,
            tc.tile_pool(name="opool", bufs=3) as opool,
            tc.tile_pool(name="ps", bufs=8, space="PSUM") as ps_pool,
        ):
            hT = hpool.tile([P, ICH, npt], bf16)

            # Startup: gate half, up half, then token chunks smallest-first in
            # ascending order. Phase A walks blocks0 in the same order, so
            # every dependency lands just before PE reaches it and the PE
            # never stalls (an idle PE resets the clock ramp to half speed).
            wg0 = wpool.tile([P, DCH * P], bf16, tag="wg")
            nc.sync.dma_start(wg0[:], wst_d[0][:, : DCH * P])
            wu0 = wpool.tile([P, DCH * P], bf16, tag="wu")
            nc.sync.dma_start(wu0[:], wst_d[0][:, DCH * P :])
            xb = {}
            for t0, n in blocks0:
                xt = xpool.tile([P, DCH, n], bf16, tag=f"xb{t0}")
                xb[t0] = xt
                nc.sync.dma_start(xt[:], xt_d[:, DCH * t0 : DCH * (t0 + n)])

            # ---- phase A: gate/up + SwiGLU, h resident ----
            for ic in range(ICH):
                if ic == 0:
                    wg, wu = wg0, wu0
                else:
                    wg = wpool.tile([P, DCH * P], bf16, tag="wg")
                    nc.sync.dma_start(wg[:], wst_d[ic][:, : DCH * P])
                    wu = wpool.tile([P, DCH * P], bf16, tag="wu")
                    nc.sync.dma_start(wu[:], wst_d[ic][:, DCH * P :])
                for t0, n in blocks0:
                    xt = xb[t0]
                    pg = ps_pool.tile([P, BLK], f32, tag="ps", name=f"pg_{ic}_{t0}")
                    pu = ps_pool.tile([P, BLK], f32, tag="ps", name=f"pu_{ic}_{t0}")
                    for dc in range(DCH):
                        nc.tensor.matmul(
                            pg[:, :n],
                            wg[:, dc * P : (dc + 1) * P],
                            xt[:, dc, :],
                            start=(dc == 0),
                            stop=(dc == DCH - 1),
                        )
                        nc.tensor.matmul(
                            pu[:, :n],
                            wu[:, dc * P : (dc + 1) * P],
                            xt[:, dc, :],
                            start=(dc == 0),
                            stop=(dc == DCH - 1),
                        )
                    sg = spool.tile([P, BLK], bf16, tag="sg")
                    nc.scalar.activation(
                        sg[:, :n], pg[:, :n], mybir.ActivationFunctionType.Silu
                    )
                    nc.vector.tensor_mul(hT[:, ic, t0 : t0 + n], sg[:, :n], pu[:, :n])

            # ---- phase B: down proj ----
            for dc in range(DCH):
                w2 = w2pool.tile([P, ICH * P], bf16, tag="w2t")
                nc.sync.dma_start(w2[:], w2t_d[dc])
                # ascending: the first down matmul then reads h written ~3us
                # before phase A ends (no stall), and the final output DMA
                # (critical path) is the tiny tail block
                for t0, n in sorted(blocks):
                    po = ps_pool.tile([P, BLK], f32, tag="ps", name=f"po_{dc}_{t0}")
                    for ic in range(ICH):
                        nc.tensor.matmul(
                            po[:, :n],
                            w2[:, ic * P : (ic + 1) * P],
                            hT[:, ic, t0 : t0 + n],
                            start=(ic == 0),
                            stop=(ic == ICH - 1),
                        )
                    ob = opool.tile([P, BLK], bf16, tag="ob")
                    nc.scalar.activation(
                        ob[:, :n], po[:, :n], mybir.ActivationFunctionType.Copy
                    )
                    nc.sync.dma_start(out_d[dc, :, t0 : t0 + n], ob[:, :n])

    nc.compile()
    return nc


def _prepare(hidden_states, router_w, ws, w2s):
    """Host-side routing, per-expert packing, weight transposes, bf16 casts."""
    import ml_dtypes

    bf16 = ml_dtypes.bfloat16

    x = np.asarray(hidden_states, dtype=np.float32).reshape(T, D)
    router_w = np.asarray(router_w, dtype=np.float32)
    ws = np.asarray(ws, dtype=np.float32)
    w2s = np.asarray(w2s, dtype=np.float32)

    top1, top2, w1, w2 = _host_router(x, router_w)

    # per-expert token lists; pos[k, t] = global packed position of token t's
    # top-k contribution (k=0 -> top1 expert, k=1 -> top2 expert)
    toks: list[list[int]] = [[] for _ in range(E)]
    loc = np.zeros((2, T), dtype=np.int64)
    exp = np.zeros((2, T), dtype=np.int64)
    for k, ti in enumerate((top1, top2)):
        for t in range(T):
            e = int(ti[t])
            loc[k, t] = len(toks[e])
            exp[k, t] = e
            toks[e].append(t)
    cnts = [len(tk) for tk in toks]
    npt = max(cnts)
    base = np.zeros(E + 1, dtype=np.int64)
    base[1:] = np.cumsum(cnts)
    pos = base[exp] + loc  # [2, T]

    x_bf = x.astype(bf16)
    sblocks = _startup_blocks(npt)
    xt_all = []
    for e in range(E):
        cnt = cnts[e]
        xe = np.zeros((npt, DCH, P), dtype=bf16)
        xe[:cnt] = x_bf[toks[e]].reshape(cnt, DCH, P)
        # chunk-major: block (t0, n) -> cols [DCH*t0, DCH*(t0+n)) as [dc, tok]
        xtc = np.empty((P, DCH * npt), dtype=bf16)
        for t0, n in sblocks:
            xtc[:, DCH * t0 : DCH * (t0 + n)] = (
                xe[t0 : t0 + n].transpose(2, 1, 0).reshape(P, DCH * n)
            )
        xt_all.append(xtc)

    wst_all = []
    w2t_all = []
    for e in range(E):
        gate = ws[e, :I, :]  # [I, D]
        up = ws[e, I:, :]
        # [ICH, P(d), DCH, P(m=i)]
        gt = gate.reshape(ICH, P, DCH, P).transpose(0, 3, 2, 1)
        ut = up.reshape(ICH, P, DCH, P).transpose(0, 3, 2, 1)
        wst = np.stack([gt, ut], axis=2)  # [ICH, P, 2, DCH, P]
        wst_all.append(
            np.ascontiguousarray(wst.reshape(ICH, P, 2 * DCH * P).astype(bf16))
        )
        # w2s[e]: [D, I] -> [DCH, P(i), ICH, P(m=d)]
        w2t = w2s[e].reshape(DCH, P, ICH, P).transpose(0, 3, 2, 1)
        w2t_all.append(
            np.ascontiguousarray(w2t.reshape(DCH, P, ICH * P).astype(bf16))
        )

    return cnts, npt, pos, (w1, w2), xt_all, wst_all, w2t_all


def kernel(hidden_states, router_w, ws, w2s):
    from concourse import bass_utils

    hs = np.asarray(hidden_states)
    B, S, _ = hs.shape
    cnts, npt, pos, (w1, w2), xt_all, wst_all, w2t_all = _prepare(
        hidden_states, router_w, ws, w2s
    )

    if npt not in _CACHE:
        _CACHE[npt] = _build_bass(npt)
    nc = _CACHE[npt]

    in_maps = [
        {"xt": xt_all[c], "wst": wst_all[c], "w2t": w2t_all[c]} for c in range(NCORES)
    ]
    res = bass_utils.run_bass_kernel_spmd(nc, in_maps, core_ids=list(range(NCORES)))

    # assemble: per-core out [DCH, P, npt] -> packed [sum(cnts), D]
    packed = np.empty((sum(cnts), D), dtype=np.float32)
    base = 0
    for e in range(E):
        oc = np.asarray(res.results[e]["out"]).astype(np.float32)  # [DCH, P, npt]
        packed[base : base + cnts[e]] = (
            oc[:, :, : cnts[e]].reshape(D, cnts[e]).T
        )
        base += cnts[e]

    out = w1[:, None] * packed[pos[0]] + w2[:, None] * packed[pos[1]]
    return out.reshape(B, S, D).astype(np.float32)
